# revision 37
# baseline (speedup 1.0000x reference)
"""Trainium2 Bass kernel for per-node masked MLP (gnn_message_passing).

Reference computation (B=8192 batch, T=128 nodes, H=64 hidden, C=2 out):
    h   = leaky_relu(einsum('tij,jt,bj->bti', w0, adj, x) + b0)   adj = 1-eye
    h   = leaky_relu(einsum('tij,btj->bti', w1, h) + b1)
    out = einsum('tij,btj->bti', w2, h) + b2

Strategy: data-parallel over batch across 8 NeuronCores (1024 rows each).
Per core, all three layers are TensorE matmuls with the (t,i) axes on PSUM
partitions and batch streaming on the moving free dim (fp32r -> full-rate
PE). All weights are preloaded into SBUF once (12 MB resident).
  L0: [j=128, ti-tile=128] stationary per 128-wide ti block (self-loop mask
      folded into the weights host-side).
  L1: block-diagonal [W1[2m].T (+) W1[2m+1].T] stationary per node pair.
  L2: 128-wide stationary accumulating 32 node pairs into one PSUM bank
      (each pair owns a distinct 4-column strip).
Bias + leaky-relu ride the PSUM->SBUF evacuation, load-balanced across
ScalarE (fused Lrelu activation) and VectorE (fused single-pass custom DVE
op LEAKY_BIAS_ANT). Evacuation is the wall-clock bottleneck (PE ~82us of
matmul columns vs ~105-115us of evacuation work split over two engines),
so the final config ("split1024") gives each engine statically-owned psum
slots and SBUF tiles: L0's psum is one [128,1024] two-bank tile evacuated
by a single wide ScalarE Lrelu; L1's two 512-wide halves alternate
ScalarE/VectorE per the a1_dve_period knob, with slot b1 always on the
fused VectorE op. Engine-disjoint destination tiles are what lets the two
engines actually overlap (a shared tile serializes them through Tile's
dependency tracking).
"""

import sys

if "/opt/trn_rl_repo" not in sys.path:
    sys.path.insert(0, "/opt/trn_rl_repo")

import numpy as np

B = 8192
T = 128
H = 64
C = 2
N_CORES = 8
BC = B // N_CORES  # 1024 batch rows per core
M_TILES = 64  # 128-wide (t,i) tiles for L0 == node pairs for L1/L2
NEG = 0.01  # leaky_relu negative slope

LEAKY_OP_NAME = "LEAKY_BIAS_ANT"


def _register_leaky_op():
    """Custom fused DVE op: out = max(in0 + s0, in0*imm2 + s1).
    With s0=bias, s1=NEG*bias, imm2=NEG this is leaky_relu(in0+bias) in one
    VectorE pass. Registered into concourse.dve_ops.OPS at runtime."""
    import concourse.dve_ops as dve_ops
    from concourse.dve_ops import DveOp
    from concourse.dve_spec import C0, C1, C2, Spec, Src0, lower, maxx
    from concourse.dve_uop import DveOpSpec

    for op in dve_ops.OPS:
        if op.name == LEAKY_OP_NAME:
            return op

    def _reference(in0, in1, s0, s1, imm2):
        z = in0.astype(np.float32)
        return np.maximum(z + s0, z * imm2 + s1).astype(np.float32)

    spec = Spec(body=maxx(Src0 + C0, Src0 * C2 + C1), reference=_reference)
    row = dve_ops._CUSTOM_DVE_ROW_BASE + len(dve_ops.OPS)
    assert row < 0x20
    shas = {}
    for ver in ("v3", "v4"):
        s = DveOpSpec(
            name=LEAKY_OP_NAME, opcode=row, uops=lower(spec, ver=ver), rd1_en=False
        )
        shas[ver] = s.sha(ver)
    op = DveOp(LEAKY_OP_NAME, spec, subdim=False, uops_sha=shas)
    dve_ops.OPS.append(op)
    dve_ops._SUB_OPCODE_FOR_NAME[LEAKY_OP_NAME] = row
    dve_ops.CUSTOM_DVE_SPECS[LEAKY_OP_NAME] = spec
    return op


def _split_sync_waits(nc, cap=1):
    """This container's walrus build encodes at most ~1 sync wait per
    instruction (setupSyncWait: "Too many sync wait commands"), while Tile's
    sem assignment freely attaches several. Post-pass: leave `cap` waits on
    each instruction and hoist the extras onto single-wait NOPs inserted
    just before it on the same engine (same-engine FIFO preserves
    semantics)."""
    from concourse import mybir

    ctr = [0]
    for f in nc.m.functions:
        for blk in f.blocks:
            new_list = []
            for ins in blk.instructions:
                si = getattr(ins, "sync_info", None)
                waits = list(si.on_wait) if si is not None and si.on_wait else []
                if len(waits) > cap:
                    keep = waits[:cap]
                    extra = waits[cap:]
                    for w in extra:
                        ctr[0] += 1
                        nop = mybir.InstNoOp(
                            name=f"{ins.name}-ws{ctr[0]}",
                            engine=ins.engine,
                            ins=[],
                            outs=[],
                            sync_info=mybir.SyncInfo(on_wait=[w], on_update=[]),
                        )
                        new_list.append(nop)
                    ins.sync_info = mybir.SyncInfo(
                        on_wait=keep, on_update=list(si.on_update or [])
                    )
                new_list.append(ins)
            blk.instructions[:] = new_list


def build_program(loop_R=None, evac="mixed", l2="wide", dve_frac=None, evac_width=512, skew=False, wait_cap=1, b0_dve_period=4, l1_tiled=False, l2_tiled=False, psa_bufs=None, psb_bufs=None, dve_custom=False, l2_evac_dve=False, a1_dve_period=None, h_bufs=2, h_bf16=False, l0_bf16=False, unroll=1):
    """Build the per-core Bass program.

    loop_R: wrap the body in a hardware For_i loop (wall-clock slope timing).
    evac:  "mixed" (ACT Lrelu + custom DVE op, balanced), "act", "dvec"
           (custom DVE only), "dve" (2-op DVE only), "none" (timing probe:
           matmuls run on stale h tiles, no evacuation work), "split"
           (slot-static ACT/DVE split with engine-disjoint SBUF tiles).
    l2:    "wide" | "off" (timing probe).
    b0_dve_period: in evac="split", slot b0 goes to DVE every k-th m
           (None = never; 1 = always). Slots a0/a1 are ACT, b1 is DVE.
    l1_tiled: L1 as two concurrent 64x64 tile_position matmuls per pair.
    l2_tiled: L2 col-tiled 4x via tile_position (32-wide stationary).
    """
    import concourse.bass as bass
    import concourse.tile as tile
    from concourse import mybir

    f32 = mybir.dt.float32
    f32r = mybir.dt.float32r
    Alu = mybir.AluOpType
    Act = mybir.ActivationFunctionType

    leaky_op = _register_leaky_op() if dve_custom else None
    bf16 = mybir.dt.bfloat16
    wdt = bf16 if h_bf16 else f32r  # w1/w2 stationary + h moving dtype
    xdt = bf16 if l0_bf16 else f32r  # x moving + w0 stationary dtype

    nc = bass.Bass()
    xt_d = nc.dram_tensor("xt", [T, BC], xdt, kind="ExternalInput")
    w0_d = nc.dram_tensor("w0w", [T, M_TILES * T], xdt, kind="ExternalInput")
    w1_d = nc.dram_tensor("w1w", [T, M_TILES * T], wdt, kind="ExternalInput")
    w2_d = nc.dram_tensor("w2w", [T, M_TILES * T], wdt, kind="ExternalInput")
    b0_d = nc.dram_tensor("b0s", [T, M_TILES], f32, kind="ExternalInput")
    b1_d = nc.dram_tensor("b1s", [T, M_TILES], f32, kind="ExternalInput")
    b1p_d = nc.dram_tensor("b1p", [T, M_TILES], f32, kind="ExternalInput")
    b0p_d = nc.dram_tensor("b0p", [T, M_TILES], f32, kind="ExternalInput")
    b2_d = nc.dram_tensor("b2s", [T, 2], f32, kind="ExternalInput")
    out_d = nc.dram_tensor("out", [2 * T, BC], f32, kind="ExternalOutput")

    with tile.TileContext(nc) as tc:
        with (
            tc.tile_pool(name="const", bufs=1) as cp,
            tc.tile_pool(name="h0p", bufs=h_bufs) as h0p,
            tc.tile_pool(name="h0q", bufs=h_bufs) as h0q,
            tc.tile_pool(name="h1p", bufs=h_bufs) as h1p,
            tc.tile_pool(name="h1q", bufs=h_bufs) as h1q,
            tc.tile_pool(name="tmp", bufs=3) as tmpp,
            tc.tile_pool(name="outp", bufs=2) as outp,
            tc.tile_pool(
                name="psA",
                bufs=(psa_bufs if psa_bufs else (4 if skew else 3)),
                space=bass.MemorySpace.PSUM,
            ) as psA,
            tc.tile_pool(
                name="psB",
                bufs=(psb_bufs if psb_bufs else (2 if skew else 3)),
                space=bass.MemorySpace.PSUM,
            ) as psB,
            tc.tile_pool(name="psCa", bufs=1, space=bass.MemorySpace.PSUM) as psCa,
            tc.tile_pool(name="psCb", bufs=1, space=bass.MemorySpace.PSUM) as psCb,
        ):
            # ---- resident tensors (loaded once) ----
            xtt = cp.tile([T, BC], xdt, tag="xt")
            nc.sync.dma_start(xtt[:], xt_d[:])
            w0sb = cp.tile([T, M_TILES * T], xdt, tag="w0w")
            nc.sync.dma_start(w0sb[:], w0_d[:])
            w1sb = cp.tile([T, M_TILES * T], wdt, tag="w1w")
            nc.sync.dma_start(w1sb[:], w1_d[:])
            w2sb = cp.tile([T, M_TILES * T], wdt, tag="w2w")
            nc.sync.dma_start(w2sb[:], w2_d[:])
            b0t = cp.tile([T, M_TILES], f32, tag="b0")
            nc.sync.dma_start(b0t[:], b0_d[:])
            b0pt = cp.tile([T, M_TILES], f32, tag="b0p")
            nc.sync.dma_start(b0pt[:], b0p_d[:])
            b1t = cp.tile([T, M_TILES], f32, tag="b1")
            nc.sync.dma_start(b1t[:], b1_d[:])
            b1pt = cp.tile([T, M_TILES], f32, tag="b1p")
            nc.sync.dma_start(b1pt[:], b1p_d[:])
            b2t = cp.tile([T, 2], f32, tag="b2")
            nc.sync.dma_start(b2t[:], b2_d[:])

            if evac == "none":
                h0fix = cp.tile([T, BC], f32r, tag="h0fix")
                nc.gpsimd.memset(h0fix[:].bitcast(f32), 0.125)
                h1fix = cp.tile([T, BC], f32r, tag="h1fix")
                nc.gpsimd.memset(h1fix[:].bitcast(f32), 0.125)
            if evac == "empty":
                scratch = cp.tile([T, 16], f32, tag="scratch")

            # ACT Lrelu ~570ns vs DVE 2-op ~1316ns per [128,512] tile:
            # give VectorE ~31% of the evacuations (4/13).
            # (A fused custom DVE op would halve the DVE cost, but this
            # walrus build rejects InstCustomDveAnt: "ISA wrong length".)
            if dve_frac is None:
                n_dve, n_mod = (4, 13) if evac_width == 512 else (3, 10)
            else:
                n_dve, n_mod = dve_frac
            ctr = [0]

            def evac_leaky(dst, ps, bias_col, bias01_col, engine=None):
                k = ctr[0]
                ctr[0] += 1
                if engine is None:
                    use_dve = (evac in ("dvec", "dve")) or (
                        evac == "mixed" and k % n_mod < n_dve
                    )
                else:
                    use_dve = engine == "dve"
                if use_dve:
                    if dve_custom:
                        nc.vector._custom_dve(
                            leaky_op, out=dst, in0=ps,
                            s0=bias_col, s1=bias01_col, imm2=NEG,
                        )
                        return
                    t1 = tmpp.tile([T, evac_width], f32, tag="t1")
                    nc.vector.tensor_scalar(
                        t1[:], ps, NEG, bias01_col, op0=Alu.mult, op1=Alu.add
                    )
                    nc.vector.scalar_tensor_tensor(
                        dst, ps, bias_col, t1[:], op0=Alu.add, op1=Alu.max
                    )
                else:
                    nc.scalar.activation(
                        dst, ps, Act.Lrelu, bias=bias_col, scale=1.0, alpha=NEG
                    )

            state = {}
            unit_ctr = [0]

            def wide_engine():
                u = unit_ctr[0]
                unit_ctr[0] += 1
                n_d, n_m = b0_dve_period if isinstance(b0_dve_period, tuple) else (3, 8)
                return "dve" if u % n_m < n_d else "act"

            def wide_evac(dst_pool_act, dst_pool_dve, ps, bias_col, bias01_col, tag):
                """One [T,1024] psum unit -> SBUF. ACT: single wide Lrelu.
                DVE: two 512-wide fused custom ops (2-bank DVE reads untested;
                two halves cost the same). Engine-disjoint SBUF pools."""
                eng = wide_engine()
                if eng == "act":
                    hF = dst_pool_act.tile([T, 1024], f32r, tag=tag + "a")
                    nc.scalar.activation(
                        hF[:], ps[:, 0:1024], Act.Lrelu,
                        bias=bias_col, scale=1.0, alpha=NEG,
                    )
                else:
                    hF = dst_pool_dve.tile([T, 1024], f32r, tag=tag + "d")
                    nc.vector._custom_dve(
                        leaky_op, out=hF[:, 0:512], in0=ps[:, 0:512],
                        s0=bias_col, s1=bias01_col, imm2=NEG,
                    )
                    nc.vector._custom_dve(
                        leaky_op, out=hF[:, 512:1024], in0=ps[:, 512:1024],
                        s0=bias_col, s1=bias01_col, imm2=NEG,
                    )
                return hF

            def l1_matmuls(ps, w1off, rhs):
                """L1 pair matmul: block-diag single MM, or two concurrent
                64x64 tile-position MMs (same math, half the PE occupancy)."""
                if l1_tiled:
                    nc.tensor.matmul(
                        ps[0:64, :], w1sb[0:64, w1off : w1off + 64], rhs[0:64, :],
                        start=True, stop=True, tile_position=(0, 0),
                    )
                    nc.tensor.matmul(
                        ps[64:128, :],
                        w1sb[64:128, w1off + 64 : w1off + 128],
                        rhs[64:128, :],
                        start=True, stop=True, tile_position=(64, 64),
                    )
                else:
                    nc.tensor.matmul(
                        ps[:], w1sb[:, w1off : w1off + 128], rhs, start=True, stop=True
                    )

            def l2_matmuls(m, mq, h1a_ap, h1b_ap):
                ps2a, ps2b = state["ps2"]
                if l2_tiled:
                    j, q = divmod(mq, 8)
                    w2t = w2sb[:, T * m + 32 * j : T * m + 32 * j + 32]
                    nc.tensor.matmul(
                        ps2a[32 * j : 32 * j + 32, :], w2t, h1a_ap,
                        start=(q == 0), stop=(q == 7), tile_position=(0, 32 * j),
                    )
                    nc.tensor.matmul(
                        ps2b[32 * j : 32 * j + 32, :], w2t, h1b_ap,
                        start=(q == 0), stop=(q == 7), tile_position=(0, 32 * j),
                    )
                else:
                    w2t = w2sb[:, T * m : T * (m + 1)]
                    nc.tensor.matmul(
                        ps2a[:], w2t, h1a_ap, start=(mq == 0), stop=(mq == 31)
                    )
                    nc.tensor.matmul(
                        ps2b[:], w2t, h1b_ap, start=(mq == 0), stop=(mq == 31)
                    )

            def stage_l0(m):
                w0t = w0sb[:, T * m : T * (m + 1)]
                if evac == "wide3":
                    ps0 = psA.tile([T, 1024], f32, tag="psW")
                    nc.tensor.matmul(
                        ps0[:, 0:512], w0t, xtt[:, 0:512], start=True, stop=True
                    )
                    nc.tensor.matmul(
                        ps0[:, 512:1024], w0t, xtt[:, 512:1024],
                        start=True, stop=True,
                    )
                    h0F = wide_evac(
                        h0p, h0q, ps0, b0t[:, m : m + 1], b0pt[:, m : m + 1], "h0"
                    )
                    state[("h0", m)] = (h0F[:, 0:512], h0F[:, 512:1024])
                    return
                if evac == "split1024":
                    ps0 = psA.tile([T, 1024], f32, tag="ps0")
                    nc.tensor.matmul(
                        ps0[:, 0:512], w0t, xtt[:, 0:512], start=True, stop=True
                    )
                    nc.tensor.matmul(
                        ps0[:, 512:1024], w0t, xtt[:, 512:1024],
                        start=True, stop=True,
                    )
                    # unit A engine: ACT by default, DVE per b0_dve_period
                    if isinstance(b0_dve_period, tuple):
                        n_d, n_m = b0_dve_period
                        eng_a = "dve" if m % n_m < n_d else "act"
                    else:
                        eng_a = (
                            "dve"
                            if (b0_dve_period and m % b0_dve_period == b0_dve_period - 1)
                            else "act"
                        )
                    h0F = h0p.tile([T, 1024], wdt, tag="h0F")
                    if eng_a == "dve" and dve_custom:
                        nc.vector._custom_dve(
                            leaky_op, out=h0F[:], in0=ps0[:, 0:1024],
                            s0=b0t[:, m : m + 1], s1=b0pt[:, m : m + 1], imm2=NEG,
                        )
                    else:
                        nc.scalar.activation(
                            h0F[:], ps0[:, 0:1024], Act.Lrelu,
                            bias=b0t[:, m : m + 1], scale=1.0, alpha=NEG,
                        )
                    state[("h0", m)] = (h0F[:, 0:512], h0F[:, 512:1024])
                    return
                if evac == "split":
                    ps0a = psA.tile([T, 512], f32, tag="ps0")
                    nc.tensor.matmul(
                        ps0a[:], w0t, xtt[:, 0:512], start=True, stop=True
                    )
                    ps0b = psA.tile([T, 512], f32, tag="ps0")
                    nc.tensor.matmul(
                        ps0b[:], w0t, xtt[:, 512:1024], start=True, stop=True
                    )
                    h0A = h0p.tile([T, 512], wdt, tag="h0A")
                    evac_leaky(
                        h0A[:], ps0a[:], b0t[:, m : m + 1], b0pt[:, m : m + 1],
                        engine="act",
                    )
                    h0B = h0q.tile([T, 512], wdt, tag="h0B")
                    if isinstance(b0_dve_period, tuple):
                        n_d, n_m = b0_dve_period
                        eng_b0 = "dve" if m % n_m < n_d else "act"
                    else:
                        eng_b0 = (
                            "dve"
                            if (b0_dve_period and m % b0_dve_period == b0_dve_period - 1)
                            else "act"
                        )
                    evac_leaky(
                        h0B[:], ps0b[:], b0t[:, m : m + 1], b0pt[:, m : m + 1],
                        engine=eng_b0,
                    )
                    state[("h0", m)] = (h0A[:], h0B[:])
                    return
                if evac_width == 1024:
                    ps0 = psA.tile([T, 1024], f32, tag="ps")
                    nc.tensor.matmul(
                        ps0[:, 0:512], w0t, xtt[:, 0:512], start=True, stop=True
                    )
                    nc.tensor.matmul(
                        ps0[:, 512:1024], w0t, xtt[:, 512:1024],
                        start=True, stop=True,
                    )
                    ps0_parts = [(ps0[:, 0:1024], slice(0, 1024))]
                else:
                    ps0a = psA.tile([T, 512], f32, tag="ps0")
                    nc.tensor.matmul(
                        ps0a[:], w0t, xtt[:, 0:512], start=True, stop=True
                    )
                    ps0b = psA.tile([T, 512], f32, tag="ps0")
                    nc.tensor.matmul(
                        ps0b[:], w0t, xtt[:, 512:1024], start=True, stop=True
                    )
                    ps0_parts = [(ps0a[:], slice(0, 512)), (ps0b[:], slice(512, 1024))]
                if evac == "none":
                    h0 = h0fix
                else:
                    h0 = h0p.tile([T, BC], f32r, tag="h0")
                    for ps_ap, sl in ps0_parts:
                        evac_leaky(h0[:, sl], ps_ap, b0t[:, m : m + 1], b0pt[:, m : m + 1])
                state[("h0", m)] = h0

            def stage_l12(m):
                g, mq = divmod(m, 32)
                h0 = state.pop(("h0", m))
                if l2 == "wide" and mq == 0:
                    ps2a = psCa.tile([T, 512], f32, tag="ps2a")
                    ps2b = psCb.tile([T, 512], f32, tag="ps2b")
                    state["ps2"] = (ps2a, ps2b)
                if evac == "wide3":
                    h0A, h0B = h0
                    ps1 = psA.tile([T, 1024], f32, tag="psW")
                    nc.tensor.matmul(
                        ps1[:, 0:512], w1sb[:, T * m : T * m + 128], h0A,
                        start=True, stop=True,
                    )
                    nc.tensor.matmul(
                        ps1[:, 512:1024], w1sb[:, T * m : T * m + 128], h0B,
                        start=True, stop=True,
                    )
                    h1F = wide_evac(
                        h1p, h1q, ps1, b1t[:, m : m + 1], b1pt[:, m : m + 1], "h1"
                    )
                    if l2 == "wide":
                        l2_matmuls(m, mq, h1F[:, 0:512], h1F[:, 512:1024])
                        if mq == 31:
                            ps2a, ps2b = state["ps2"]
                            oa = outp.tile([T, 512], f32, tag="oa")
                            nc.scalar.activation(
                                oa[:], ps2a[:], Act.Identity, bias=b2t[:, g : g + 1]
                            )
                            nc.sync.dma_start(
                                out_d[128 * g : 128 * (g + 1), 0:512], oa[:]
                            )
                            ob = outp.tile([T, 512], f32, tag="ob")
                            nc.scalar.activation(
                                ob[:], ps2b[:], Act.Identity, bias=b2t[:, g : g + 1]
                            )
                            nc.sync.dma_start(
                                out_d[128 * g : 128 * (g + 1), 512:1024], ob[:]
                            )
                    return
                if evac in ("split", "split1024"):
                    h0A, h0B = h0
                    ps1a = psB.tile([T, 512], f32, tag="ps1")
                    l1_matmuls(ps1a, T * m, h0A)
                    ps1b = psB.tile([T, 512], f32, tag="ps1")
                    l1_matmuls(ps1b, T * m, h0B)
                    eng_a1 = "act"
                    if a1_dve_period and m % a1_dve_period == a1_dve_period - 1:
                        eng_a1 = "dve"
                    h1A = h1p.tile([T, 512], wdt, tag="h1A")
                    evac_leaky(
                        h1A[:], ps1a[:], b1t[:, m : m + 1], b1pt[:, m : m + 1],
                        engine=eng_a1,
                    )
                    h1B = h1q.tile([T, 512], wdt, tag="h1B")
                    evac_leaky(
                        h1B[:], ps1b[:], b1t[:, m : m + 1], b1pt[:, m : m + 1],
                        engine="dve",
                    )
                    if l2 == "wide":
                        l2_matmuls(m, mq, h1A[:], h1B[:])
                        if mq == 31:
                            ps2a, ps2b = state["ps2"]

                            def evac_out(dst, ps):
                                if l2_evac_dve and dve_custom:
                                    # max(ps+b2, ps*1+b2) == ps+b2
                                    nc.vector._custom_dve(
                                        leaky_op, out=dst, in0=ps,
                                        s0=b2t[:, g : g + 1],
                                        s1=b2t[:, g : g + 1], imm2=1.0,
                                    )
                                else:
                                    nc.scalar.activation(
                                        dst, ps, Act.Identity,
                                        bias=b2t[:, g : g + 1],
                                    )

                            oa = outp.tile([T, 512], f32, tag="oa")
                            evac_out(oa[:], ps2a[:])
                            nc.sync.dma_start(
                                out_d[128 * g : 128 * (g + 1), 0:512], oa[:]
                            )
                            ob = outp.tile([T, 512], f32, tag="ob")
                            evac_out(ob[:], ps2b[:])
                            nc.sync.dma_start(
                                out_d[128 * g : 128 * (g + 1), 512:1024], ob[:]
                            )
                    return
                w1t = w1sb[:, T * m : T * (m + 1)]
                if evac_width == 1024:
                    ps1 = psA.tile([T, 1024], f32, tag="ps")
                    nc.tensor.matmul(
                        ps1[:, 0:512], w1t, h0[:, 0:512], start=True, stop=True
                    )
                    nc.tensor.matmul(
                        ps1[:, 512:1024], w1t, h0[:, 512:1024],
                        start=True, stop=True,
                    )
                    ps1_parts = [(ps1[:, 0:1024], slice(0, 1024))]
                else:
                    ps1a = psB.tile([T, 512], f32, tag="ps1")
                    nc.tensor.matmul(
                        ps1a[:], w1t, h0[:, 0:512], start=True, stop=True
                    )
                    ps1b = psB.tile([T, 512], f32, tag="ps1")
                    nc.tensor.matmul(
                        ps1b[:], w1t, h0[:, 512:1024], start=True, stop=True
                    )
                    ps1_parts = [(ps1a[:], slice(0, 512)), (ps1b[:], slice(512, 1024))]
                if evac == "none":
                    h1 = h1fix
                else:
                    h1 = h1p.tile([T, BC], f32r, tag="h1")
                    for ps_ap, sl in ps1_parts:
                        evac_leaky(h1[:, sl], ps_ap, b1t[:, m : m + 1], b1pt[:, m : m + 1])
                if l2 == "wide":
                    l2_matmuls(m, mq, h1[:, 0:512], h1[:, 512:1024])
                    if mq == 31:
                        ps2a, ps2b = state["ps2"]
                        oa = outp.tile([T, 512], f32, tag="oa")
                        nc.scalar.activation(
                            oa[:], ps2a[:], Act.Identity, bias=b2t[:, g : g + 1]
                        )
                        nc.sync.dma_start(
                            out_d[128 * g : 128 * (g + 1), 0:512], oa[:]
                        )
                        ob = outp.tile([T, 512], f32, tag="ob")
                        nc.scalar.activation(
                            ob[:], ps2b[:], Act.Identity, bias=b2t[:, g : g + 1]
                        )
                        nc.sync.dma_start(
                            out_d[128 * g : 128 * (g + 1), 512:1024], ob[:]
                        )

            def body(_iv=None):
                if evac == "empty":
                    nc.gpsimd.memset(scratch[:], 0.0)
                    return
                if skew:
                    depth = skew if isinstance(skew, int) and skew > 1 else 1
                    for m in range(M_TILES + depth):
                        if m < M_TILES:
                            stage_l0(m)
                        if m >= depth:
                            stage_l12(m - depth)
                else:
                    for m in range(M_TILES):
                        stage_l0(m)
                        stage_l12(m)

            if loop_R is None:
                body()
            else:
                # Tile's For_i pays a full cross-engine barrier + all-engine
                # pipeline drain per trip (sem reset). Unrolling u bodies per
                # trip amortizes it; loop_R still means loop_R full kernel
                # executions.
                u = unroll if (unroll and loop_R % unroll == 0) else 1
                with tc.For_i(0, loop_R // u, 1) as iv:
                    for _ in range(u):
                        body(iv)

            if evac == "empty" or l2 == "off":
                # timing probes never write out_d in the body; bind it so the
                # output tensor isn't dead
                z = cp.tile([T, 16], f32, tag="zpad")
                nc.gpsimd.memset(z[:], 0.0)
                nc.sync.dma_start(out_d[0:T, 0:16], z[:])

    _split_sync_waits(nc, cap=wait_cap)
    if dve_custom:
        # Populate .instr bytes for InstCustomDveAnt — raw Bass skips the
        # Bacc codegen pass; without this walrus sees empty .instr ("ISA
        # wrong length").
        from concourse.library_overlay import lower_extended_insts

        lower_extended_insts(nc)
    return nc


def prep_inputs(x, w0, b0, w1, b1, w2, b2, h_bf16=None, l0_bf16=None):
    """Host-side reshuffle of the full inputs into the per-core tensors.
    h_bf16: emit w1w/w2w as bfloat16 (must match the program's dtype);
    defaults to BEST_CONFIG's setting."""
    if h_bf16 is None:
        h_bf16 = BEST_CONFIG.get("h_bf16", False)
    if l0_bf16 is None:
        l0_bf16 = BEST_CONFIG.get("l0_bf16", False)
    x = np.ascontiguousarray(np.asarray(x, dtype=np.float32))
    w0 = np.asarray(w0, dtype=np.float32)
    b0 = np.asarray(b0, dtype=np.float32)
    w1 = np.asarray(w1, dtype=np.float32)
    b1 = np.asarray(b1, dtype=np.float32)
    w2 = np.asarray(w2, dtype=np.float32)
    b2 = np.asarray(b2, dtype=np.float32)

    # L0 stationaries: mask self-loop; [j, (m p)] with column 128m+p -> ti
    w0m = w0.copy()
    w0m[np.arange(T), :, np.arange(T)] = 0.0
    w0w = np.ascontiguousarray(w0m.transpose(2, 0, 1).reshape(T, T * H))

    # L1 stationaries: block-diag of the pair's transposed weights
    w1T = w1.transpose(0, 2, 1)  # [t, i_in, i_out]
    w1s = np.zeros((M_TILES, T, T), np.float32)
    w1s[:, :H, :H] = w1T[0::2]
    w1s[:, H:, H:] = w1T[1::2]
    w1w = np.ascontiguousarray(w1s.transpose(1, 0, 2).reshape(T, M_TILES * T))

    # L2 stationaries: pair m owns columns 4*(m%32) .. +4
    w2T = w2.transpose(0, 2, 1)  # [t, i, c]
    w2s = np.zeros((M_TILES, T, T), np.float32)
    for m in range(M_TILES):
        col = 4 * (m % 32)
        w2s[m, :H, col : col + C] = w2T[2 * m]
        w2s[m, H:, col + C : col + 2 * C] = w2T[2 * m + 1]
    w2w = np.ascontiguousarray(w2s.transpose(1, 0, 2).reshape(T, M_TILES * T))

    b0s = np.ascontiguousarray(b0.reshape(-1).reshape(M_TILES, T).T)
    b1s = np.ascontiguousarray(b1.reshape(-1).reshape(M_TILES, T).T)
    b2s = np.ascontiguousarray(b2.reshape(-1).reshape(2, T).T)

    if h_bf16 or l0_bf16:
        from concourse import mybir

        bf = mybir.dt.np(mybir.dt.bfloat16)
        if h_bf16:
            w1w = w1w.astype(bf)
            w2w = w2w.astype(bf)
        if l0_bf16:
            w0w = w0w.astype(bf)
            x = x.astype(bf)
    shared = {
        "w0w": w0w, "w1w": w1w, "w2w": w2w,
        "b0s": b0s, "b0p": np.ascontiguousarray(NEG * b0s),
        "b1s": b1s, "b1p": np.ascontiguousarray(NEG * b1s),
        "b2s": b2s,
    }
    in_maps = []
    for c in range(N_CORES):
        xt_c = np.ascontiguousarray(x[c * BC : (c + 1) * BC].T)  # [128, BC]
        in_maps.append({"xt": xt_c, **shared})
    return in_maps


def gather_output(results):
    """results: list of per-core {"out": [256, BC]} -> full [B, T, C]."""
    parts = []
    for c in range(N_CORES):
        o = np.asarray(results[c]["out"])  # [2T, BC], row r = t*2+c
        parts.append(o.reshape(T, C, BC).transpose(2, 0, 1))
    return np.ascontiguousarray(np.concatenate(parts, axis=0))


_NC_CACHE = {}


# Measured per-iter (min-slope, R=64 vs 512, resident-input PJRT runner):
#   all-ACT evac (old best):                 190.8 us
#   split ACT/DVE 2-op, shared ratio knobs:  150-184 us
#   split + fused custom DVE op (1 op/tile): 136.4 us  (b0 every 2nd to DVE)
#   split + custom op, b0 2-of-3 to DVE:     123.2 us  <- BEST
#   split1024 (2-bank ACT evac, no crash):   123.6 us
# Key fixes vs the previous session: (1) engine-disjoint SBUF h tiles per
# evacuation engine (shared tiles serialized ACT/DVE via Tile dependency
# tracking); (2) InstCustomDveAnt needs lower_extended_insts(nc) after
# build -- the earlier "ISA wrong length" was the missing codegen pass, not
# a walrus limitation. tile_position array tiling is NOT supported by this
# walrus (s3d3_mm_valid_dst_partition on any nonzero tile). GPSIMD has no
# PSUM port; DMA has no PSUM route -- ACT+DVE are the only evac lanes.
# Final: split1024 = L0 psum as one [T,1024] 2-bank tile evacuated by a
# single wide ACT Lrelu (2-bank PSUM reads are safe on this runtime -- the
# prior session's crash was unrelated); L1 halves split ACT/DVE per
# a1_dve_period, slot b1 always on the fused DVE op.
#   split  (2,3) 512-wide:  123.2 / 127.0 / 130.8 us across runs
#   split1024 a1=2:         123.6 / 122.4 / 125.4 / 118.6 us  <- BEST
#   wide3 (L0+L1 all-wide, shared 3-buf [T,1024] pool): 138.1 us -- worse;
#     the shared pool couples L0/L1 psum reuse to both engines' evac
#     latencies and stalls the PE.
#   skew=2 (L0 issued 2 m's ahead, psa4/psb2 512-wide): 181.0 us -- much
#     worse; psb_bufs=2 gives the DVE-paced ps1 lane only 1-m reuse slack.
#     skew-2 with psb=3 cannot fit the 8-bank PSUM budget.
# Lrelu+Identity share the single real gen3 act-table set containing Lrelu
# (checked get_activation_tables) -- no table-load thrash at g boundaries.
# Further neighborhood probes (all equal or worse than BEST_CONFIG):
#   split1024 + skew=2 + h_bufs=3:    122.9 us (neutral -- L0->L1 evac slack
#     was not the binding stall)
#   psa_bufs=1 / psb_bufs=4:          139.7 us (single-buffered L0 psum
#     serializes L0 matmuls behind the wide ACT evac)
#   b0=(1,4) wide unit to DVE, a1 ACT-always: 157.0 us (wide 2-bank DVE
#     custom ops are disproportionately slow -- PSUM has 1 DVE read port)
#   h_bf16 (h/w1/w2 in bfloat16, rel err 3.6e-3): 135.4 us -- worse; no
#     ScalarE 2-byte-output accel on this HW, and the bf16 matmul/downcast
#     path costs more than the narrower data saves.
# bf16 adoption (same-window A/B/A, fp32 brackets at 117.8/126.2):
#   h_bf16 (h, w1, w2 bf16):       112.2 us, rel err 3.6e-3
#   + l0_bf16 (x, w0 bf16 too):    110.8 us, rel err 3.9e-3  <- BEST
# bf16 stationaries qualify for Fast Weight Load (f32r does not:
# EnableFWL needs in_dtype != FP32 + reads 2 bf16/32-bit) -- halves
# LDWEIGHTS, shrinking PE-induced stalls. An earlier "bf16 slower" reading
# (135 us) was host-interference contamination; same-window A/B/A settled it.
# unroll=4: Tile's For_i pays a cross-engine barrier + all-engine pipeline
# drain per trip (block of 11 InstDrain + 12 InstEventSemaphore inside the
# loop, ~7.6us/body at unroll=1). Four bodies per trip amortize it:
# same-window A/B/A measured 104.9 us vs 110.5/110.8 at unroll=1, and
# unroll=8 at 102.8 vs 104.3/104.1 at unroll=4.
# loop_R still means loop_R full kernel executions; single-shot unaffected.
BEST_CONFIG = dict(
    evac="split1024", skew=True, psa_bufs=2, psb_bufs=2,
    dve_custom=True, b0_dve_period=None, a1_dve_period=2,
    h_bf16=True, l0_bf16=True, unroll=8,
)


def kernel(x, w0, b0, w1, b1, w2, b2):
    from concourse.bass_utils import run_bass_kernel_spmd

    if "nc" not in _NC_CACHE:
        _NC_CACHE["nc"] = build_program(**BEST_CONFIG)
    nc = _NC_CACHE["nc"]
    in_maps = prep_inputs(x, w0, b0, w1, b1, w2, b2)
    res = run_bass_kernel_spmd(nc, in_maps, core_ids=list(range(N_CORES)))
    return gather_output(res.results)



# revision 38
# speedup vs baseline: 1.0173x; 1.0173x over previous
"""Trainium2 Bass kernel for per-node masked MLP (gnn_message_passing).

Reference computation (B=8192 batch, T=128 nodes, H=64 hidden, C=2 out):
    h   = leaky_relu(einsum('tij,jt,bj->bti', w0, adj, x) + b0)   adj = 1-eye
    h   = leaky_relu(einsum('tij,btj->bti', w1, h) + b1)
    out = einsum('tij,btj->bti', w2, h) + b2

Strategy: data-parallel over batch across 8 NeuronCores (1024 rows each).
Per core, all three layers are TensorE matmuls with the (t,i) axes on PSUM
partitions and batch streaming on the moving free dim (fp32r -> full-rate
PE). All weights are preloaded into SBUF once (12 MB resident).
  L0: [j=128, ti-tile=128] stationary per 128-wide ti block (self-loop mask
      folded into the weights host-side).
  L1: block-diagonal [W1[2m].T (+) W1[2m+1].T] stationary per node pair.
  L2: 128-wide stationary accumulating 32 node pairs into one PSUM bank
      (each pair owns a distinct 4-column strip).
Bias + leaky-relu ride the PSUM->SBUF evacuation, load-balanced across
ScalarE (fused Lrelu activation) and VectorE (fused single-pass custom DVE
op LEAKY_BIAS_ANT). Evacuation is the wall-clock bottleneck (PE ~82us of
matmul columns vs ~105-115us of evacuation work split over two engines),
so the final config ("split1024") gives each engine statically-owned psum
slots and SBUF tiles: L0's psum is one [128,1024] two-bank tile evacuated
by a single wide ScalarE Lrelu; L1's two 512-wide halves alternate
ScalarE/VectorE per the a1_dve_period knob, with slot b1 always on the
fused VectorE op. Engine-disjoint destination tiles are what lets the two
engines actually overlap (a shared tile serializes them through Tile's
dependency tracking).
"""

import sys

if "/opt/trn_rl_repo" not in sys.path:
    sys.path.insert(0, "/opt/trn_rl_repo")

import numpy as np

B = 8192
T = 128
H = 64
C = 2
N_CORES = 8
BC = B // N_CORES  # 1024 batch rows per core
M_TILES = 64  # 128-wide (t,i) tiles for L0 == node pairs for L1/L2
NEG = 0.01  # leaky_relu negative slope

LEAKY_OP_NAME = "LEAKY_BIAS_ANT"


def _register_leaky_op():
    """Custom fused DVE op: out = max(in0 + s0, in0*imm2 + s1).
    With s0=bias, s1=NEG*bias, imm2=NEG this is leaky_relu(in0+bias) in one
    VectorE pass. Registered into concourse.dve_ops.OPS at runtime."""
    import concourse.dve_ops as dve_ops
    from concourse.dve_ops import DveOp
    from concourse.dve_spec import C0, C1, C2, Spec, Src0, lower, maxx
    from concourse.dve_uop import DveOpSpec

    for op in dve_ops.OPS:
        if op.name == LEAKY_OP_NAME:
            return op

    def _reference(in0, in1, s0, s1, imm2):
        z = in0.astype(np.float32)
        return np.maximum(z + s0, z * imm2 + s1).astype(np.float32)

    spec = Spec(body=maxx(Src0 + C0, Src0 * C2 + C1), reference=_reference)
    row = dve_ops._CUSTOM_DVE_ROW_BASE + len(dve_ops.OPS)
    assert row < 0x20
    shas = {}
    for ver in ("v3", "v4"):
        s = DveOpSpec(
            name=LEAKY_OP_NAME, opcode=row, uops=lower(spec, ver=ver), rd1_en=False
        )
        shas[ver] = s.sha(ver)
    op = DveOp(LEAKY_OP_NAME, spec, subdim=False, uops_sha=shas)
    dve_ops.OPS.append(op)
    dve_ops._SUB_OPCODE_FOR_NAME[LEAKY_OP_NAME] = row
    dve_ops.CUSTOM_DVE_SPECS[LEAKY_OP_NAME] = spec
    return op


def _split_sync_waits(nc, cap=1):
    """This container's walrus build encodes at most ~1 sync wait per
    instruction (setupSyncWait: "Too many sync wait commands"), while Tile's
    sem assignment freely attaches several. Post-pass: leave `cap` waits on
    each instruction and hoist the extras onto single-wait NOPs inserted
    just before it on the same engine (same-engine FIFO preserves
    semantics)."""
    from concourse import mybir

    ctr = [0]
    for f in nc.m.functions:
        for blk in f.blocks:
            new_list = []
            for ins in blk.instructions:
                si = getattr(ins, "sync_info", None)
                waits = list(si.on_wait) if si is not None and si.on_wait else []
                if len(waits) > cap:
                    keep = waits[:cap]
                    extra = waits[cap:]
                    for w in extra:
                        ctr[0] += 1
                        nop = mybir.InstNoOp(
                            name=f"{ins.name}-ws{ctr[0]}",
                            engine=ins.engine,
                            ins=[],
                            outs=[],
                            sync_info=mybir.SyncInfo(on_wait=[w], on_update=[]),
                        )
                        new_list.append(nop)
                    ins.sync_info = mybir.SyncInfo(
                        on_wait=keep, on_update=list(si.on_update or [])
                    )
                new_list.append(ins)
            blk.instructions[:] = new_list


def build_program(loop_R=None, evac="mixed", l2="wide", dve_frac=None, evac_width=512, skew=False, wait_cap=1, b0_dve_period=4, l1_tiled=False, l2_tiled=False, psa_bufs=None, psb_bufs=None, dve_custom=False, l2_evac_dve=False, a1_dve_period=None, h_bufs=2, h_bf16=False, l0_bf16=False, unroll=1):
    """Build the per-core Bass program.

    loop_R: wrap the body in a hardware For_i loop (wall-clock slope timing).
    evac:  "mixed" (ACT Lrelu + custom DVE op, balanced), "act", "dvec"
           (custom DVE only), "dve" (2-op DVE only), "none" (timing probe:
           matmuls run on stale h tiles, no evacuation work), "split"
           (slot-static ACT/DVE split with engine-disjoint SBUF tiles).
    l2:    "wide" | "off" (timing probe).
    b0_dve_period: in evac="split", slot b0 goes to DVE every k-th m
           (None = never; 1 = always). Slots a0/a1 are ACT, b1 is DVE.
    l1_tiled: L1 as two concurrent 64x64 tile_position matmuls per pair.
    l2_tiled: L2 col-tiled 4x via tile_position (32-wide stationary).
    """
    import concourse.bass as bass
    import concourse.tile as tile
    from concourse import mybir

    f32 = mybir.dt.float32
    f32r = mybir.dt.float32r
    Alu = mybir.AluOpType
    Act = mybir.ActivationFunctionType

    leaky_op = _register_leaky_op() if dve_custom else None
    bf16 = mybir.dt.bfloat16
    wdt = bf16 if h_bf16 else f32r  # w1/w2 stationary + h moving dtype
    xdt = bf16 if l0_bf16 else f32r  # x moving + w0 stationary dtype

    nc = bass.Bass()
    xt_d = nc.dram_tensor("xt", [T, BC], xdt, kind="ExternalInput")
    w0_d = nc.dram_tensor("w0w", [T, M_TILES * T], xdt, kind="ExternalInput")
    w1_d = nc.dram_tensor("w1w", [T, M_TILES * T], wdt, kind="ExternalInput")
    w2_d = nc.dram_tensor("w2w", [T, M_TILES * T], wdt, kind="ExternalInput")
    b0_d = nc.dram_tensor("b0s", [T, M_TILES], f32, kind="ExternalInput")
    b1_d = nc.dram_tensor("b1s", [T, M_TILES], f32, kind="ExternalInput")
    b1p_d = nc.dram_tensor("b1p", [T, M_TILES], f32, kind="ExternalInput")
    b0p_d = nc.dram_tensor("b0p", [T, M_TILES], f32, kind="ExternalInput")
    b2_d = nc.dram_tensor("b2s", [T, 2], f32, kind="ExternalInput")
    out_d = nc.dram_tensor("out", [2 * T, BC], f32, kind="ExternalOutput")

    with tile.TileContext(nc) as tc:
        with (
            tc.tile_pool(name="const", bufs=1) as cp,
            tc.tile_pool(name="h0p", bufs=h_bufs) as h0p,
            tc.tile_pool(name="h0q", bufs=h_bufs) as h0q,
            tc.tile_pool(name="h1p", bufs=h_bufs) as h1p,
            tc.tile_pool(name="h1q", bufs=h_bufs) as h1q,
            tc.tile_pool(name="tmp", bufs=3) as tmpp,
            tc.tile_pool(name="outp", bufs=2) as outp,
            tc.tile_pool(
                name="psA",
                bufs=(psa_bufs if psa_bufs else (4 if skew else 3)),
                space=bass.MemorySpace.PSUM,
            ) as psA,
            tc.tile_pool(
                name="psB",
                bufs=(psb_bufs if psb_bufs else (2 if skew else 3)),
                space=bass.MemorySpace.PSUM,
            ) as psB,
            tc.tile_pool(name="psCa", bufs=1, space=bass.MemorySpace.PSUM) as psCa,
            tc.tile_pool(name="psCb", bufs=1, space=bass.MemorySpace.PSUM) as psCb,
        ):
            # ---- resident tensors (loaded once) ----
            xtt = cp.tile([T, BC], xdt, tag="xt")
            nc.sync.dma_start(xtt[:], xt_d[:])
            w0sb = cp.tile([T, M_TILES * T], xdt, tag="w0w")
            nc.sync.dma_start(w0sb[:], w0_d[:])
            w1sb = cp.tile([T, M_TILES * T], wdt, tag="w1w")
            nc.sync.dma_start(w1sb[:], w1_d[:])
            w2sb = cp.tile([T, M_TILES * T], wdt, tag="w2w")
            nc.sync.dma_start(w2sb[:], w2_d[:])
            b0t = cp.tile([T, M_TILES], f32, tag="b0")
            nc.sync.dma_start(b0t[:], b0_d[:])
            b0pt = cp.tile([T, M_TILES], f32, tag="b0p")
            nc.sync.dma_start(b0pt[:], b0p_d[:])
            b1t = cp.tile([T, M_TILES], f32, tag="b1")
            nc.sync.dma_start(b1t[:], b1_d[:])
            b1pt = cp.tile([T, M_TILES], f32, tag="b1p")
            nc.sync.dma_start(b1pt[:], b1p_d[:])
            b2t = cp.tile([T, 2], f32, tag="b2")
            nc.sync.dma_start(b2t[:], b2_d[:])

            if evac == "none":
                h0fix = cp.tile([T, BC], f32r, tag="h0fix")
                nc.gpsimd.memset(h0fix[:].bitcast(f32), 0.125)
                h1fix = cp.tile([T, BC], f32r, tag="h1fix")
                nc.gpsimd.memset(h1fix[:].bitcast(f32), 0.125)
            if evac == "empty":
                scratch = cp.tile([T, 16], f32, tag="scratch")

            # ACT Lrelu ~570ns vs DVE 2-op ~1316ns per [128,512] tile:
            # give VectorE ~31% of the evacuations (4/13).
            # (A fused custom DVE op would halve the DVE cost, but this
            # walrus build rejects InstCustomDveAnt: "ISA wrong length".)
            if dve_frac is None:
                n_dve, n_mod = (4, 13) if evac_width == 512 else (3, 10)
            else:
                n_dve, n_mod = dve_frac
            ctr = [0]

            def evac_leaky(dst, ps, bias_col, bias01_col, engine=None):
                k = ctr[0]
                ctr[0] += 1
                if engine is None:
                    use_dve = (evac in ("dvec", "dve")) or (
                        evac == "mixed" and k % n_mod < n_dve
                    )
                else:
                    use_dve = engine == "dve"
                if use_dve:
                    if dve_custom:
                        nc.vector._custom_dve(
                            leaky_op, out=dst, in0=ps,
                            s0=bias_col, s1=bias01_col, imm2=NEG,
                        )
                        return
                    t1 = tmpp.tile([T, evac_width], f32, tag="t1")
                    nc.vector.tensor_scalar(
                        t1[:], ps, NEG, bias01_col, op0=Alu.mult, op1=Alu.add
                    )
                    nc.vector.scalar_tensor_tensor(
                        dst, ps, bias_col, t1[:], op0=Alu.add, op1=Alu.max
                    )
                else:
                    nc.scalar.activation(
                        dst, ps, Act.Lrelu, bias=bias_col, scale=1.0, alpha=NEG
                    )

            state = {}
            unit_ctr = [0]

            def wide_engine():
                u = unit_ctr[0]
                unit_ctr[0] += 1
                n_d, n_m = b0_dve_period if isinstance(b0_dve_period, tuple) else (3, 8)
                return "dve" if u % n_m < n_d else "act"

            def wide_evac(dst_pool_act, dst_pool_dve, ps, bias_col, bias01_col, tag):
                """One [T,1024] psum unit -> SBUF. ACT: single wide Lrelu.
                DVE: two 512-wide fused custom ops (2-bank DVE reads untested;
                two halves cost the same). Engine-disjoint SBUF pools."""
                eng = wide_engine()
                if eng == "act":
                    hF = dst_pool_act.tile([T, 1024], f32r, tag=tag + "a")
                    nc.scalar.activation(
                        hF[:], ps[:, 0:1024], Act.Lrelu,
                        bias=bias_col, scale=1.0, alpha=NEG,
                    )
                else:
                    hF = dst_pool_dve.tile([T, 1024], f32r, tag=tag + "d")
                    nc.vector._custom_dve(
                        leaky_op, out=hF[:, 0:512], in0=ps[:, 0:512],
                        s0=bias_col, s1=bias01_col, imm2=NEG,
                    )
                    nc.vector._custom_dve(
                        leaky_op, out=hF[:, 512:1024], in0=ps[:, 512:1024],
                        s0=bias_col, s1=bias01_col, imm2=NEG,
                    )
                return hF

            def l1_matmuls(ps, w1off, rhs):
                """L1 pair matmul: block-diag single MM, or two concurrent
                64x64 tile-position MMs (same math, half the PE occupancy)."""
                if l1_tiled:
                    nc.tensor.matmul(
                        ps[0:64, :], w1sb[0:64, w1off : w1off + 64], rhs[0:64, :],
                        start=True, stop=True, tile_position=(0, 0),
                    )
                    nc.tensor.matmul(
                        ps[64:128, :],
                        w1sb[64:128, w1off + 64 : w1off + 128],
                        rhs[64:128, :],
                        start=True, stop=True, tile_position=(64, 64),
                    )
                else:
                    nc.tensor.matmul(
                        ps[:], w1sb[:, w1off : w1off + 128], rhs, start=True, stop=True
                    )

            def l2_matmuls(m, mq, h1a_ap, h1b_ap):
                ps2a, ps2b = state["ps2"]
                if l2_tiled:
                    j, q = divmod(mq, 8)
                    w2t = w2sb[:, T * m + 32 * j : T * m + 32 * j + 32]
                    nc.tensor.matmul(
                        ps2a[32 * j : 32 * j + 32, :], w2t, h1a_ap,
                        start=(q == 0), stop=(q == 7), tile_position=(0, 32 * j),
                    )
                    nc.tensor.matmul(
                        ps2b[32 * j : 32 * j + 32, :], w2t, h1b_ap,
                        start=(q == 0), stop=(q == 7), tile_position=(0, 32 * j),
                    )
                else:
                    w2t = w2sb[:, T * m : T * (m + 1)]
                    nc.tensor.matmul(
                        ps2a[:], w2t, h1a_ap, start=(mq == 0), stop=(mq == 31)
                    )
                    nc.tensor.matmul(
                        ps2b[:], w2t, h1b_ap, start=(mq == 0), stop=(mq == 31)
                    )

            def stage_l0(m):
                w0t = w0sb[:, T * m : T * (m + 1)]
                if evac == "wide3":
                    ps0 = psA.tile([T, 1024], f32, tag="psW")
                    nc.tensor.matmul(
                        ps0[:, 0:512], w0t, xtt[:, 0:512], start=True, stop=True
                    )
                    nc.tensor.matmul(
                        ps0[:, 512:1024], w0t, xtt[:, 512:1024],
                        start=True, stop=True,
                    )
                    h0F = wide_evac(
                        h0p, h0q, ps0, b0t[:, m : m + 1], b0pt[:, m : m + 1], "h0"
                    )
                    state[("h0", m)] = (h0F[:, 0:512], h0F[:, 512:1024])
                    return
                if evac == "split1024":
                    ps0 = psA.tile([T, 1024], f32, tag="ps0")
                    nc.tensor.matmul(
                        ps0[:, 0:512], w0t, xtt[:, 0:512], start=True, stop=True
                    )
                    nc.tensor.matmul(
                        ps0[:, 512:1024], w0t, xtt[:, 512:1024],
                        start=True, stop=True,
                    )
                    # unit A engine: ACT by default, DVE per b0_dve_period
                    if isinstance(b0_dve_period, tuple):
                        n_d, n_m = b0_dve_period
                        eng_a = "dve" if m % n_m < n_d else "act"
                    else:
                        eng_a = (
                            "dve"
                            if (b0_dve_period and m % b0_dve_period == b0_dve_period - 1)
                            else "act"
                        )
                    h0F = h0p.tile([T, 1024], wdt, tag="h0F")
                    if eng_a == "dve" and dve_custom:
                        nc.vector._custom_dve(
                            leaky_op, out=h0F[:], in0=ps0[:, 0:1024],
                            s0=b0t[:, m : m + 1], s1=b0pt[:, m : m + 1], imm2=NEG,
                        )
                    else:
                        nc.scalar.activation(
                            h0F[:], ps0[:, 0:1024], Act.Lrelu,
                            bias=b0t[:, m : m + 1], scale=1.0, alpha=NEG,
                        )
                    state[("h0", m)] = (h0F[:, 0:512], h0F[:, 512:1024])
                    return
                if evac == "split":
                    ps0a = psA.tile([T, 512], f32, tag="ps0")
                    nc.tensor.matmul(
                        ps0a[:], w0t, xtt[:, 0:512], start=True, stop=True
                    )
                    ps0b = psA.tile([T, 512], f32, tag="ps0")
                    nc.tensor.matmul(
                        ps0b[:], w0t, xtt[:, 512:1024], start=True, stop=True
                    )
                    h0A = h0p.tile([T, 512], wdt, tag="h0A")
                    evac_leaky(
                        h0A[:], ps0a[:], b0t[:, m : m + 1], b0pt[:, m : m + 1],
                        engine="act",
                    )
                    h0B = h0q.tile([T, 512], wdt, tag="h0B")
                    if isinstance(b0_dve_period, tuple):
                        n_d, n_m = b0_dve_period
                        eng_b0 = "dve" if m % n_m < n_d else "act"
                    else:
                        eng_b0 = (
                            "dve"
                            if (b0_dve_period and m % b0_dve_period == b0_dve_period - 1)
                            else "act"
                        )
                    evac_leaky(
                        h0B[:], ps0b[:], b0t[:, m : m + 1], b0pt[:, m : m + 1],
                        engine=eng_b0,
                    )
                    state[("h0", m)] = (h0A[:], h0B[:])
                    return
                if evac_width == 1024:
                    ps0 = psA.tile([T, 1024], f32, tag="ps")
                    nc.tensor.matmul(
                        ps0[:, 0:512], w0t, xtt[:, 0:512], start=True, stop=True
                    )
                    nc.tensor.matmul(
                        ps0[:, 512:1024], w0t, xtt[:, 512:1024],
                        start=True, stop=True,
                    )
                    ps0_parts = [(ps0[:, 0:1024], slice(0, 1024))]
                else:
                    ps0a = psA.tile([T, 512], f32, tag="ps0")
                    nc.tensor.matmul(
                        ps0a[:], w0t, xtt[:, 0:512], start=True, stop=True
                    )
                    ps0b = psA.tile([T, 512], f32, tag="ps0")
                    nc.tensor.matmul(
                        ps0b[:], w0t, xtt[:, 512:1024], start=True, stop=True
                    )
                    ps0_parts = [(ps0a[:], slice(0, 512)), (ps0b[:], slice(512, 1024))]
                if evac == "none":
                    h0 = h0fix
                else:
                    h0 = h0p.tile([T, BC], f32r, tag="h0")
                    for ps_ap, sl in ps0_parts:
                        evac_leaky(h0[:, sl], ps_ap, b0t[:, m : m + 1], b0pt[:, m : m + 1])
                state[("h0", m)] = h0

            def stage_l12(m):
                g, mq = divmod(m, 32)
                h0 = state.pop(("h0", m))
                if l2 == "wide" and mq == 0:
                    ps2a = psCa.tile([T, 512], f32, tag="ps2a")
                    ps2b = psCb.tile([T, 512], f32, tag="ps2b")
                    state["ps2"] = (ps2a, ps2b)
                if evac == "wide3":
                    h0A, h0B = h0
                    ps1 = psA.tile([T, 1024], f32, tag="psW")
                    nc.tensor.matmul(
                        ps1[:, 0:512], w1sb[:, T * m : T * m + 128], h0A,
                        start=True, stop=True,
                    )
                    nc.tensor.matmul(
                        ps1[:, 512:1024], w1sb[:, T * m : T * m + 128], h0B,
                        start=True, stop=True,
                    )
                    h1F = wide_evac(
                        h1p, h1q, ps1, b1t[:, m : m + 1], b1pt[:, m : m + 1], "h1"
                    )
                    if l2 == "wide":
                        l2_matmuls(m, mq, h1F[:, 0:512], h1F[:, 512:1024])
                        if mq == 31:
                            ps2a, ps2b = state["ps2"]
                            oa = outp.tile([T, 512], f32, tag="oa")
                            nc.scalar.activation(
                                oa[:], ps2a[:], Act.Identity, bias=b2t[:, g : g + 1]
                            )
                            nc.sync.dma_start(
                                out_d[128 * g : 128 * (g + 1), 0:512], oa[:]
                            )
                            ob = outp.tile([T, 512], f32, tag="ob")
                            nc.scalar.activation(
                                ob[:], ps2b[:], Act.Identity, bias=b2t[:, g : g + 1]
                            )
                            nc.sync.dma_start(
                                out_d[128 * g : 128 * (g + 1), 512:1024], ob[:]
                            )
                    return
                if evac in ("split", "split1024"):
                    h0A, h0B = h0
                    ps1a = psB.tile([T, 512], f32, tag="ps1")
                    l1_matmuls(ps1a, T * m, h0A)
                    ps1b = psB.tile([T, 512], f32, tag="ps1")
                    l1_matmuls(ps1b, T * m, h0B)
                    eng_a1 = "act"
                    if isinstance(a1_dve_period, tuple):
                        n_d, n_m = a1_dve_period
                        if m % n_m < n_d:
                            eng_a1 = "dve"
                    elif a1_dve_period and m % a1_dve_period == a1_dve_period - 1:
                        eng_a1 = "dve"
                    h1A = h1p.tile([T, 512], wdt, tag="h1A")
                    evac_leaky(
                        h1A[:], ps1a[:], b1t[:, m : m + 1], b1pt[:, m : m + 1],
                        engine=eng_a1,
                    )
                    h1B = h1q.tile([T, 512], wdt, tag="h1B")
                    evac_leaky(
                        h1B[:], ps1b[:], b1t[:, m : m + 1], b1pt[:, m : m + 1],
                        engine="dve",
                    )
                    if l2 == "wide":
                        l2_matmuls(m, mq, h1A[:], h1B[:])
                        if mq == 31:
                            ps2a, ps2b = state["ps2"]

                            def evac_out(dst, ps, half_dve=False):
                                use_dve = l2_evac_dve and dve_custom
                                if l2_evac_dve == "half":
                                    use_dve = half_dve and dve_custom
                                if use_dve:
                                    # max(ps+b2, ps*1+b2) == ps+b2
                                    nc.vector._custom_dve(
                                        leaky_op, out=dst, in0=ps,
                                        s0=b2t[:, g : g + 1],
                                        s1=b2t[:, g : g + 1], imm2=1.0,
                                    )
                                else:
                                    nc.scalar.activation(
                                        dst, ps, Act.Identity,
                                        bias=b2t[:, g : g + 1],
                                    )

                            oa = outp.tile([T, 512], f32, tag="oa")
                            evac_out(oa[:], ps2a[:])
                            nc.sync.dma_start(
                                out_d[128 * g : 128 * (g + 1), 0:512], oa[:]
                            )
                            ob = outp.tile([T, 512], f32, tag="ob")
                            evac_out(ob[:], ps2b[:], half_dve=True)
                            nc.sync.dma_start(
                                out_d[128 * g : 128 * (g + 1), 512:1024], ob[:]
                            )
                    return
                w1t = w1sb[:, T * m : T * (m + 1)]
                if evac_width == 1024:
                    ps1 = psA.tile([T, 1024], f32, tag="ps")
                    nc.tensor.matmul(
                        ps1[:, 0:512], w1t, h0[:, 0:512], start=True, stop=True
                    )
                    nc.tensor.matmul(
                        ps1[:, 512:1024], w1t, h0[:, 512:1024],
                        start=True, stop=True,
                    )
                    ps1_parts = [(ps1[:, 0:1024], slice(0, 1024))]
                else:
                    ps1a = psB.tile([T, 512], f32, tag="ps1")
                    nc.tensor.matmul(
                        ps1a[:], w1t, h0[:, 0:512], start=True, stop=True
                    )
                    ps1b = psB.tile([T, 512], f32, tag="ps1")
                    nc.tensor.matmul(
                        ps1b[:], w1t, h0[:, 512:1024], start=True, stop=True
                    )
                    ps1_parts = [(ps1a[:], slice(0, 512)), (ps1b[:], slice(512, 1024))]
                if evac == "none":
                    h1 = h1fix
                else:
                    h1 = h1p.tile([T, BC], f32r, tag="h1")
                    for ps_ap, sl in ps1_parts:
                        evac_leaky(h1[:, sl], ps_ap, b1t[:, m : m + 1], b1pt[:, m : m + 1])
                if l2 == "wide":
                    l2_matmuls(m, mq, h1[:, 0:512], h1[:, 512:1024])
                    if mq == 31:
                        ps2a, ps2b = state["ps2"]
                        oa = outp.tile([T, 512], f32, tag="oa")
                        nc.scalar.activation(
                            oa[:], ps2a[:], Act.Identity, bias=b2t[:, g : g + 1]
                        )
                        nc.sync.dma_start(
                            out_d[128 * g : 128 * (g + 1), 0:512], oa[:]
                        )
                        ob = outp.tile([T, 512], f32, tag="ob")
                        nc.scalar.activation(
                            ob[:], ps2b[:], Act.Identity, bias=b2t[:, g : g + 1]
                        )
                        nc.sync.dma_start(
                            out_d[128 * g : 128 * (g + 1), 512:1024], ob[:]
                        )

            def body(_iv=None):
                if evac == "empty":
                    nc.gpsimd.memset(scratch[:], 0.0)
                    return
                if skew:
                    depth = skew if isinstance(skew, int) and skew > 1 else 1
                    for m in range(M_TILES + depth):
                        if m < M_TILES:
                            stage_l0(m)
                        if m >= depth:
                            stage_l12(m - depth)
                else:
                    for m in range(M_TILES):
                        stage_l0(m)
                        stage_l12(m)

            if loop_R is None:
                body()
            else:
                # Tile's For_i pays a full cross-engine barrier + all-engine
                # pipeline drain per trip (sem reset). Unrolling u bodies per
                # trip amortizes it; loop_R still means loop_R full kernel
                # executions.
                u = unroll if (unroll and loop_R % unroll == 0) else 1
                with tc.For_i(0, loop_R // u, 1) as iv:
                    for _ in range(u):
                        body(iv)

            if evac == "empty" or l2 == "off":
                # timing probes never write out_d in the body; bind it so the
                # output tensor isn't dead
                z = cp.tile([T, 16], f32, tag="zpad")
                nc.gpsimd.memset(z[:], 0.0)
                nc.sync.dma_start(out_d[0:T, 0:16], z[:])

    _split_sync_waits(nc, cap=wait_cap)
    if dve_custom:
        # Populate .instr bytes for InstCustomDveAnt — raw Bass skips the
        # Bacc codegen pass; without this walrus sees empty .instr ("ISA
        # wrong length").
        from concourse.library_overlay import lower_extended_insts

        lower_extended_insts(nc)
    return nc


def prep_inputs(x, w0, b0, w1, b1, w2, b2, h_bf16=None, l0_bf16=None):
    """Host-side reshuffle of the full inputs into the per-core tensors.
    h_bf16: emit w1w/w2w as bfloat16 (must match the program's dtype);
    defaults to BEST_CONFIG's setting."""
    if h_bf16 is None:
        h_bf16 = BEST_CONFIG.get("h_bf16", False)
    if l0_bf16 is None:
        l0_bf16 = BEST_CONFIG.get("l0_bf16", False)
    x = np.ascontiguousarray(np.asarray(x, dtype=np.float32))
    w0 = np.asarray(w0, dtype=np.float32)
    b0 = np.asarray(b0, dtype=np.float32)
    w1 = np.asarray(w1, dtype=np.float32)
    b1 = np.asarray(b1, dtype=np.float32)
    w2 = np.asarray(w2, dtype=np.float32)
    b2 = np.asarray(b2, dtype=np.float32)

    # L0 stationaries: mask self-loop; [j, (m p)] with column 128m+p -> ti
    w0m = w0.copy()
    w0m[np.arange(T), :, np.arange(T)] = 0.0
    w0w = np.ascontiguousarray(w0m.transpose(2, 0, 1).reshape(T, T * H))

    # L1 stationaries: block-diag of the pair's transposed weights
    w1T = w1.transpose(0, 2, 1)  # [t, i_in, i_out]
    w1s = np.zeros((M_TILES, T, T), np.float32)
    w1s[:, :H, :H] = w1T[0::2]
    w1s[:, H:, H:] = w1T[1::2]
    w1w = np.ascontiguousarray(w1s.transpose(1, 0, 2).reshape(T, M_TILES * T))

    # L2 stationaries: pair m owns columns 4*(m%32) .. +4
    w2T = w2.transpose(0, 2, 1)  # [t, i, c]
    w2s = np.zeros((M_TILES, T, T), np.float32)
    for m in range(M_TILES):
        col = 4 * (m % 32)
        w2s[m, :H, col : col + C] = w2T[2 * m]
        w2s[m, H:, col + C : col + 2 * C] = w2T[2 * m + 1]
    w2w = np.ascontiguousarray(w2s.transpose(1, 0, 2).reshape(T, M_TILES * T))

    b0s = np.ascontiguousarray(b0.reshape(-1).reshape(M_TILES, T).T)
    b1s = np.ascontiguousarray(b1.reshape(-1).reshape(M_TILES, T).T)
    b2s = np.ascontiguousarray(b2.reshape(-1).reshape(2, T).T)

    if h_bf16 or l0_bf16:
        from concourse import mybir

        bf = mybir.dt.np(mybir.dt.bfloat16)
        if h_bf16:
            w1w = w1w.astype(bf)
            w2w = w2w.astype(bf)
        if l0_bf16:
            w0w = w0w.astype(bf)
            x = x.astype(bf)
    shared = {
        "w0w": w0w, "w1w": w1w, "w2w": w2w,
        "b0s": b0s, "b0p": np.ascontiguousarray(NEG * b0s),
        "b1s": b1s, "b1p": np.ascontiguousarray(NEG * b1s),
        "b2s": b2s,
    }
    in_maps = []
    for c in range(N_CORES):
        xt_c = np.ascontiguousarray(x[c * BC : (c + 1) * BC].T)  # [128, BC]
        in_maps.append({"xt": xt_c, **shared})
    return in_maps


def gather_output(results):
    """results: list of per-core {"out": [256, BC]} -> full [B, T, C]."""
    parts = []
    for c in range(N_CORES):
        o = np.asarray(results[c]["out"])  # [2T, BC], row r = t*2+c
        parts.append(o.reshape(T, C, BC).transpose(2, 0, 1))
    return np.ascontiguousarray(np.concatenate(parts, axis=0))


_NC_CACHE = {}


# Measured per-iter (min-slope, R=64 vs 512, resident-input PJRT runner):
#   all-ACT evac (old best):                 190.8 us
#   split ACT/DVE 2-op, shared ratio knobs:  150-184 us
#   split + fused custom DVE op (1 op/tile): 136.4 us  (b0 every 2nd to DVE)
#   split + custom op, b0 2-of-3 to DVE:     123.2 us  <- BEST
#   split1024 (2-bank ACT evac, no crash):   123.6 us
# Key fixes vs the previous session: (1) engine-disjoint SBUF h tiles per
# evacuation engine (shared tiles serialized ACT/DVE via Tile dependency
# tracking); (2) InstCustomDveAnt needs lower_extended_insts(nc) after
# build -- the earlier "ISA wrong length" was the missing codegen pass, not
# a walrus limitation. tile_position array tiling is NOT supported by this
# walrus (s3d3_mm_valid_dst_partition on any nonzero tile). GPSIMD has no
# PSUM port; DMA has no PSUM route -- ACT+DVE are the only evac lanes.
# Final: split1024 = L0 psum as one [T,1024] 2-bank tile evacuated by a
# single wide ACT Lrelu (2-bank PSUM reads are safe on this runtime -- the
# prior session's crash was unrelated); L1 halves split ACT/DVE per
# a1_dve_period, slot b1 always on the fused DVE op.
#   split  (2,3) 512-wide:  123.2 / 127.0 / 130.8 us across runs
#   split1024 a1=2:         123.6 / 122.4 / 125.4 / 118.6 us  <- BEST
#   wide3 (L0+L1 all-wide, shared 3-buf [T,1024] pool): 138.1 us -- worse;
#     the shared pool couples L0/L1 psum reuse to both engines' evac
#     latencies and stalls the PE.
#   skew=2 (L0 issued 2 m's ahead, psa4/psb2 512-wide): 181.0 us -- much
#     worse; psb_bufs=2 gives the DVE-paced ps1 lane only 1-m reuse slack.
#     skew-2 with psb=3 cannot fit the 8-bank PSUM budget.
# Lrelu+Identity share the single real gen3 act-table set containing Lrelu
# (checked get_activation_tables) -- no table-load thrash at g boundaries.
# Further neighborhood probes (all equal or worse than BEST_CONFIG):
#   split1024 + skew=2 + h_bufs=3:    122.9 us (neutral -- L0->L1 evac slack
#     was not the binding stall)
#   psa_bufs=1 / psb_bufs=4:          139.7 us (single-buffered L0 psum
#     serializes L0 matmuls behind the wide ACT evac)
#   b0=(1,4) wide unit to DVE, a1 ACT-always: 157.0 us (wide 2-bank DVE
#     custom ops are disproportionately slow -- PSUM has 1 DVE read port)
#   h_bf16 (h/w1/w2 in bfloat16, rel err 3.6e-3): 135.4 us -- worse; no
#     ScalarE 2-byte-output accel on this HW, and the bf16 matmul/downcast
#     path costs more than the narrower data saves.
# bf16 adoption (same-window A/B/A, fp32 brackets at 117.8/126.2):
#   h_bf16 (h, w1, w2 bf16):       112.2 us, rel err 3.6e-3
#   + l0_bf16 (x, w0 bf16 too):    110.8 us, rel err 3.9e-3  <- BEST
# bf16 stationaries qualify for Fast Weight Load (f32r does not:
# EnableFWL needs in_dtype != FP32 + reads 2 bf16/32-bit) -- halves
# LDWEIGHTS, shrinking PE-induced stalls. An earlier "bf16 slower" reading
# (135 us) was host-interference contamination; same-window A/B/A settled it.
# unroll=4: Tile's For_i pays a cross-engine barrier + all-engine pipeline
# drain per trip (block of 11 InstDrain + 12 InstEventSemaphore inside the
# loop, ~7.6us/body at unroll=1). Four bodies per trip amortize it:
# same-window A/B/A measured 104.9 us vs 110.5/110.8 at unroll=1, and
# unroll=8 at 102.8 vs 104.3/104.1 at unroll=4.
# loop_R still means loop_R full kernel executions; single-shot unaffected.
BEST_CONFIG = dict(
    evac="split1024", skew=True, psa_bufs=2, psb_bufs=2,
    dve_custom=True, b0_dve_period=None, a1_dve_period=2,
    h_bf16=True, l0_bf16=True, unroll=8,
)


def kernel(x, w0, b0, w1, b1, w2, b2):
    from concourse.bass_utils import run_bass_kernel_spmd

    if "nc" not in _NC_CACHE:
        _NC_CACHE["nc"] = build_program(**BEST_CONFIG)
    nc = _NC_CACHE["nc"]
    in_maps = prep_inputs(x, w0, b0, w1, b1, w2, b2)
    res = run_bass_kernel_spmd(nc, in_maps, core_ids=list(range(N_CORES)))
    return gather_output(res.results)

